# revision 2
# baseline (speedup 1.0000x reference)
"""DoSRUconv Trainium2 kernel, fp16 rework of the fp32r baseline.

Sharding: H (dim 3) split across 8 cores, 16 rows each; the 3x3x3 conv halo
is resolved host-side by handing each core a zero-padded 18-row slab in fp16,
so cores are fully independent (no collectives).

Differences vs the fp32r baseline (same overall dataflow):
  - everything on-chip is fp16 except PSUM accumulation (fp32): halves DMA
    bytes, halves PE transpose cost, enables DVE 2x/4x modes.
  - taps regrouped dw-major so the 8 shifted x replicas load in 3 DMAs
    (partition dim spans (dh, ci), dh via overlapping-window src strides);
    the (1,1)-tap leftover block loads in 1 DMA (dt via overlapping t
    strides).  Full-T replica tiles (no half-T split): 6 DMAs per (b,chunk)
    vs 16 before.
  - gates staged pixel-major as [128w, 96 (6g x 16c), 32 t-slots] with the
    t-slot packed innermost; PE transposes write a [col, t]-shaped PSUM tile
    so the scatter and all elementwise ops run with unit-stride fp16
    operands (DVE 2x/4x eligible).
  - elementwise epilogue as scalar_tensor_tensor ops (2 ALU ops/inst, DVE
    2x_2p/4x_2p) instead of gpsimd tensor_tensor.
  - scan runs on a (c-outer, slot-inner) strided view; recurrence state is
    fp32 inside the DVE scan op regardless of operand dtype.
  - PSUM evictions split ACT/DVE by a knob; conv matmuls ordered dt-outer
    within each 4-t PSUM group so stationary weights reload 4x less often.
  - output written [b, w, h, c, t] fp16, one DMA per (b,chunk); host
    transposes back to [B, C, T, H, W] fp32.
"""

import os

import numpy as np

import concourse.bass as bass
import concourse.mybir as mybir
import concourse.tile as tile
from concourse import bacc
from concourse.bass_utils import run_bass_kernel_spmd
from concourse.masks import make_identity

F32 = mybir.dt.float32
F16 = mybir.dt.float16
ALU = mybir.AluOpType
ACTF = mybir.ActivationFunctionType

B, CIN, COUT, T, H, W = 2, 16, 16, 31, 128, 128
NCORES = 8
HSLAB = H // NCORES                  # 16
HC = 2                               # h-rows per conv chunk (matmul N = HC*128)
NCHUNK = HSLAB // HC                 # 8
TP, WP = T + 2, W + 2                # padded dims: 33 t-slices, 130 w
SLOTS = 32                           # t-slots per channel segment (31 t + sep)
NPIX = HC * W                        # 256
RPITCH = 264                         # replica slot pitch: 2*130 + 4 slack
GCOLS = 96                           # gate columns (6 gates x 16 ch)

# taps grouped dw-major: partition p = (dw-group, dh, ci); leftover is (1,1)
TAPS = [(dh, dw) for dw in (-1, 0, 1) for dh in (-1, 0, 1)][:8]
# dma groups: (p0, ndh, dw)
TAP_DMA_GROUPS = [(0, 3, -1), (48, 3, 0), (96, 2, 1)]
# gate order [Wx, X, ft, rt, ft2, rt2] from reference split order
# [Wx, ft, ft2, rt, rt2, X]
GPERM = [0, 5, 1, 3, 2, 4]
# column base per gate in the pixel-major store
C_WX, C_X, C_FT, C_RT, C_FT2, C_RT2 = 0, 16, 32, 48, 64, 80

# xs slab strides (elements): [B, 3(dh), CIN, TP, HSLAB+2, WP]
S_H = WP                              # 130
S_T = (HSLAB + 2) * WP                # 2340
S_CI = TP * S_T                       # 77220
S_D = CIN * S_CI
S_B = 3 * S_D


def _ap(t_ap, off, dims):
    return bass.AP(tensor=t_ap.tensor, offset=t_ap.offset + off,
                   ap=[list(d) for d in dims])


def _rhs_view(rep, slot):
    """Matmul rhs [P, HC, W] from a replica tile [P, slots, RPITCH]:
    element (r, w) at slot*RPITCH + 2 + 130*r + w."""
    base = rep[:, slot]          # [P, RPITCH]
    return _ap(base, 2, [base.ap[0], [S_H, HC], [1, W]])


def _rev_slot(tile_ap, col0, ncol, start, count, nslots=SLOTS):
    """View [128, ncol, count] of a [128, cols, slots] tile with the slot dim
    (innermost) reversed, descending from `start`."""
    return _ap(tile_ap, col0 * nslots + start,
               [tile_ap.ap[0], [nslots, ncol], [-1, count]])


def _flat2(ap):
    return ap.rearrange("p a b -> p (a b)")


def build_nc():
    nc = bacc.Bacc("TRN2", target_bir_lowering=False, debug=False)

    # xs[b, d] is the 18-row slab shifted down by (d-1) rows: the (dh, ci)
    # contraction rows of each tap-group DMA land at one uniform stride.
    xs = nc.dram_tensor("xs", [B, 3, CIN, TP, HSLAB + 2, WP], F16,
                        kind="ExternalInput").ap()
    wmain_d = nc.dram_tensor("wmain", [128, 3, GCOLS], F16,
                             kind="ExternalInput").ap()
    wleft_d = nc.dram_tensor("wleft", [49, GCOLS], F16,
                             kind="ExternalInput").ap()
    ones_d = nc.dram_tensor("ones", [1, T * RPITCH], F16,
                            kind="ExternalInput").ap()
    # stored [b, w, h, c, t]; host transposes back to [b, c, t, h, w]
    out_d = nc.dram_tensor("out", [B, W, HSLAB, COUT, T], F16,
                           kind="ExternalOutput").ap()

    ev_pat = os.environ.get("K2_EVICT", "AAD")   # cycle over psum groups
    with tile.TileContext(nc) as tc:
        with (
            tc.tile_pool(name="const", bufs=1) as constp,
            tc.tile_pool(name="mrep", bufs=int(os.environ.get("K2_MREP_BUFS", "2"))) as mpool,
            tc.tile_pool(name="lrep", bufs=2) as lpool,
            tc.tile_pool(name="gc", bufs=int(os.environ.get("K2_GC_BUFS", "2"))) as gcpool,
            tc.tile_pool(name="gp", bufs=int(os.environ.get("K2_GP_BUFS", "2"))) as gppool,
            tc.tile_pool(name="scr", bufs=2) as scrp,
            tc.tile_pool(name="ot", bufs=2) as otpool,
            tc.tile_pool(name="mmps", bufs=int(os.environ.get("K2_MM_BUFS", "2")), space="PSUM") as mmpool,
            tc.tile_pool(name="trps", bufs=int(os.environ.get("K2_TR_BUFS", "2")), space="PSUM") as trpool,
        ):
            wmain_sb = constp.tile([128, 3, GCOLS], F16)
            nc.sync.dma_start(out=wmain_sb, in_=wmain_d)
            wleft_sb = constp.tile([49, GCOLS], F16)
            nc.sync.dma_start(out=wleft_sb, in_=wleft_d)
            ident = constp.tile([128, 128], F16)
            make_identity(nc, ident)

            _lim = int(os.environ.get("K2_CHUNK_LIMIT", "0"))
            _pairs = [(b, c) for b in range(B) for c in range(NCHUNK)]
            if _lim:
                _pairs = _pairs[:_lim]
            for b, chunk in _pairs:
                h0 = chunk * HC

                # ---- replica loads: 3 tap-group DMAs + 1 leftover + ones
                mrep = mpool.tile([128, TP, RPITCH], F16, tag="mrep")
                for (p0, ndh, dw) in TAP_DMA_GROUPS:
                    off = b * S_B + (h0 + 1) * S_H
                    nc.sync.dma_start(
                        out=mrep[p0:p0 + 16 * ndh, :, 1 - dw:1 - dw + 2 * WP],
                        in_=_ap(xs, off, [[S_CI, 16 * ndh],
                                          [S_T, TP], [1, 2 * WP]]))
                lrep = lpool.tile([49, T, RPITCH], F16, tag="lrep")
                for dtg in range(3):
                    nc.sync.dma_start(
                        out=lrep[dtg * 16:(dtg + 1) * 16, :, 0:2 * WP],
                        in_=_ap(xs, b * S_B + 2 * S_D + dtg * S_T
                                + (h0 + 1) * S_H,
                                [[S_CI, CIN], [S_T, T], [1, 2 * WP]]))
                nc.sync.dma_start(
                    out=lrep[48:49].rearrange("p a b -> p (a b)"),
                    in_=ones_d[:, :T * RPITCH])

                gc = gcpool.tile([GCOLS, SLOTS, NPIX], F16, tag="gc")
                gp = [gppool.tile([128, GCOLS, SLOTS], F16, tag=f"gp{r}",
                                  name=f"gp{r}_{b}_{chunk}")
                      for r in range(HC)]
                for r in range(HC):
                    nc.gpsimd.memset(gp[r][:, :, SLOTS - 1], 0.0)
                ot = otpool.tile([128, HC, COUT, T], F16, tag="ot")

                # ---- conv matmuls, dt-outer within 4-t psum groups
                for gi, tg in enumerate(range(0, T, 4)):
                    gs = min(4, T - tg)
                    ps = mmpool.tile([GCOLS, 4, NPIX], F32, tag="mm")
                    for tt in range(tg, tg + gs):
                        for dt in range(3):
                            nc.tensor.matmul(
                                ps[:, tt - tg], wmain_sb[:, dt],
                                _rhs_view(mrep, tt + dt),
                                start=(dt == 0), stop=False)
                        nc.tensor.matmul(
                            ps[:, tt - tg], wleft_sb, _rhs_view(lrep, tt),
                            start=False, stop=True)
                    dst = gc[:, tg:tg + gs, :]
                    src = ps[:, :gs, :]
                    if ev_pat[gi % len(ev_pat)] == "A":
                        nc.scalar.activation(dst, src, ACTF.Copy)
                    else:
                        nc.vector.tensor_copy(dst, src)

                # ---- transpose to pixel-major + scatter (fwd + rev'd bwd)
                for cg in range(0, T, 8):
                    cs = min(8, T - cg)
                    for r in range(HC):
                        trp = trpool.tile([128, 8, 128], F16, tag="trp")
                        for j in range(cs):
                            nc.tensor.transpose(
                                trp[:, j, 0:GCOLS],
                                gc[:, cg + j, r * W:(r + 1) * W],
                                ident[:GCOLS, :GCOLS])
                        nc.vector.tensor_copy(
                            gp[r][:, 0:C_FT2, cg:cg + cs],
                            trp[:, 0:cs, 0:C_FT2].rearrange("p t c -> p c t"))
                        bwd_dst = _rev_slot(gp[r], C_FT2, 32, 30 - cg, cs)
                        bwd_src = trp[:, 0:cs, C_FT2:GCOLS].rearrange(
                            "p t c -> p c t")
                        if os.environ.get("K2_BWD", "V") == "S":
                            nc.scalar.activation(bwd_dst, bwd_src, ACTF.Copy)
                        else:
                            nc.vector.tensor_copy(bwd_dst, bwd_src)

                # ---- activations + scan + epilogue per h-row
                for r in range(HC):
                    gpr = gp[r]
                    tv = _flat2(gpr[:, 0:C_FT, :])
                    nc.scalar.activation(tv, tv, ACTF.Tanh)
                    sv = _flat2(gpr[:, C_FT:GCOLS, :])
                    nc.scalar.activation(sv, sv, ACTF.Sigmoid)
                    # re-zero the f/f2 separator slots (sigmoid(0) = 0.5)
                    nc.gpsimd.memset(gpr[:, C_FT:C_FT + 16, SLOTS - 1], 0.0)
                    nc.gpsimd.memset(gpr[:, C_FT2:C_FT2 + 16, SLOTS - 1], 0.0)

                    cf = scrp.tile([128, 16, SLOTS], F16, tag="cf")
                    cb = scrp.tile([128, 16, SLOTS], F16, tag="cb")
                    for (fc, rev_wx, cdst) in ((C_FT, False, cf),
                                               (C_FT2, True, cb)):
                        f_v = gpr[:, fc:fc + 16, :]
                        bb = scrp.tile([128, 16, SLOTS], F16, tag="bb")
                        nc.vector.tensor_scalar_sub(
                            bb[:, :, 0:1], f_v[:, :, 0:1], 1.0)
                        wx_in = (_rev_slot(gpr, C_WX, 16, 29, 30)
                                 if rev_wx else gpr[:, C_WX:C_WX + 16, 1:T])
                        nc.vector.scalar_tensor_tensor(
                            out=bb[:, :, 1:T], in0=f_v[:, :, 1:T],
                            scalar=1.0, in1=wx_in,
                            op0=ALU.subtract, op1=ALU.mult)
                        nc.gpsimd.memset(bb[:, :, SLOTS - 1], 0.0)
                        nc.vector.tensor_tensor_scan(
                            out=_flat2(cdst), data0=_flat2(f_v),
                            data1=_flat2(bb),
                            initial=0.0, op0=ALU.mult, op1=ALU.subtract)

                    s1 = scrp.tile([128, 16, T], F16, tag="s1")
                    s2 = scrp.tile([128, 16, T], F16, tag="s2")
                    dd = scrp.tile([128, 16, T], F16, tag="dd")
                    ee = scrp.tile([128, 16, T], F16, tag="ee")
                    rt_v = gpr[:, C_RT:C_RT + 16, 0:T]
                    x_v = gpr[:, C_X:C_X + 16, 0:T]
                    if os.environ.get("K2_EPI", "TT") == "TT":
                        tt = lambda out, in0, in1, op: nc.vector.tensor_tensor(
                            out=out, in0=in0, in1=in1, op=op)
                    else:
                        tt = lambda out, in0, in1, op: \
                            nc.vector.scalar_tensor_tensor(
                                out=out, in0=in0, scalar=1.0, in1=in1,
                                op0=ALU.mult, op1=op)
                    tt(s1, rt_v, cf[:, :, 0:T], ALU.mult)
                    tt(s2, gpr[:, C_RT2:C_RT2 + 16, 0:T], cb[:, :, 0:T],
                       ALU.mult)
                    tt(dd, rt_v, _rev_slot(gpr, C_RT2, 16, 30, T), ALU.add)
                    nc.vector.scalar_tensor_tensor(
                        out=ee, in0=dd, scalar=2.0, in1=x_v,
                        op0=ALU.subtract, op1=ALU.mult)
                    tt(s1, s1, _rev_slot(s2, 0, 16, 30, T, T), ALU.add)
                    tt(ot[:, r], s1, ee, ALU.subtract)

                nc.scalar.dma_start(out=out_d[b, :, h0:h0 + HC], in_=ot)
    nc.compile()
    return nc


_NC_CACHE = None


def _get_nc():
    global _NC_CACHE
    if _NC_CACHE is None:
        _NC_CACHE = build_nc()
    return _NC_CACHE


def make_host_inputs(x, conv_w, conv_b):
    """Pad x, permute/flatten weights, all in fp16."""
    x = np.asarray(x, np.float32)
    conv_w = np.asarray(conv_w, np.float32)
    conv_b = np.asarray(conv_b, np.float32)

    xp = np.zeros((B, CIN, TP, H + 4, WP), np.float16)
    xp[:, :, 1:1 + T, 2:2 + H, 1:1 + W] = x

    wp = conv_w.reshape(6, COUT, CIN, 3, 3, 3)[GPERM].reshape(
        GCOLS, CIN, 3, 3, 3)
    bp = conv_b.reshape(6, COUT)[GPERM].reshape(GCOLS)

    wmain = np.zeros((128, 3, GCOLS), np.float16)
    for g, (dh, dw) in enumerate(TAPS):
        for dt in range(3):
            wmain[g * 16:(g + 1) * 16, dt, :] = wp[:, :, dt, dh + 1, dw + 1].T
    wleft = np.zeros((49, GCOLS), np.float16)
    for dtg in range(3):
        wleft[dtg * 16:(dtg + 1) * 16, :] = wp[:, :, dtg, 2, 2].T
    wleft[48, :] = bp
    ones = np.ones((1, T * RPITCH), np.float16)
    return xp, wmain, wleft, ones


def core_inputs(xp, wmain, wleft, ones, k):
    base = k * HSLAB
    xr = np.stack([xp[:, :, :, base + d:base + d + HSLAB + 2, :]
                   for d in range(3)], axis=1)
    return {
        "xs": np.ascontiguousarray(xr),
        "wmain": wmain,
        "wleft": wleft,
        "ones": ones,
    }


def kernel(x, conv_w, conv_b):
    nc = _get_nc()
    xp, wmain, wleft, ones = make_host_inputs(x, conv_w, conv_b)
    in_maps = [core_inputs(xp, wmain, wleft, ones, k) for k in range(NCORES)]
    res = run_bass_kernel_spmd(nc, in_maps, list(range(NCORES)))
    # res out: [B, W, HSLAB, C, T] per core -> [B, C, T, H, W] fp32
    outs = [res.results[k]["out"].transpose(0, 3, 4, 2, 1)
            for k in range(NCORES)]
    return np.concatenate(outs, axis=3).astype(np.float32)
